# revision 37
# baseline (speedup 1.0000x reference)
"""Trainium2 Bass kernel for nn_D_Attention_82377472738015.

Transformer decoder block: causal self-attention + cross-attention + FFN,
each with residual + layernorm.  B=8, S=1024, D=512, H=8, HD=64, DFF=2048.

Sharding: data-parallel over batch.  8 batch elements -> 8 NeuronCores,
weights replicated, no collectives.  Each core runs the full block on its
[1024, 512] slice.

Per-core design:
- Activations kept TRANSPOSED ([feature-on-partitions, tokens-free]) so every
  projection is lhsT.T @ rhs with the feature dim contracting on partitions.
- Attention scores computed transposed ([kv, q]); the two heads of a pair
  (base partitions 0/64) write into one 2-bank PSUM tile [128, 2, 512] so
  mask-add and exp run once per (pair, kv-tile) at double width.
- Causal tiles restrict all work (scores matmul, mask, exp, PV) to the
  unmasked q-range; the triangular mask touches only the 128-wide diagonal
  block.
- Q/K/V projection chunks are emitted interleaved with softmax head-pair
  loops so the scalar engine starts exp'ing ~10us into the kernel and the
  tensor engine back-fills projection matmuls during ACT-bound stretches.
- Softmax denominator via a ones-column appended to V (row 65 of the PV
  accumulator).  PV PSUM banks are evacuated with a single copy; the divide
  runs off the critical path (reciprocal on DVE, partition-broadcast on
  gpsimd), so the next head-pair's PV matmuls start immediately.
- rsqrt for layernorm as exp(-0.5*ln(var+eps)): keeps every activation
  (Exp/Ln/Relu) in the single `natural_log_exp_and_others` table set, so no
  ACT table reloads.  LN elementwise in bf16 (2x DVE rate).
- Bias+residual epilogues fused into one scalar_tensor_tensor per tile;
  V bias folded into the PSUM->SBUF move.
- bf16 matmul inputs (fp32 PSUM accumulation); bf16 output DMA split per
  512-token chunk (host upcasts).
"""

import sys

sys.path.insert(0, "/opt/trn_rl_repo")

import os
from contextlib import ExitStack

import numpy as np
import ml_dtypes

import concourse.bass as bass
import concourse.tile as tile
from concourse import bacc, mybir
from concourse.bass_utils import run_bass_kernel_spmd

# Pin every activation to the natural_log_exp_and_others table set (it holds
# exp+ln+relu — the only funcs this kernel uses) so the ACT engine never
# reloads tables mid-kernel.  Set indices are preserved; we only hide
# exp/ln/relu from the other sets so the placement pass can't pick them.
_orig_get_act_tables = bacc.get_activation_tables


def _pinned_act_tables(arch):
    tabs = dict(_orig_get_act_tables(arch))
    pin = "natural_log_exp_and_others"
    if pin in tabs:
        AF = mybir.ActivationFunctionType
        hide = {AF.Exp, AF.Ln, AF.Relu}
        hide = {f for f in hide if f in tabs[pin]}
        tabs = {
            name: (funcs if name == pin else set(funcs) - hide)
            for name, funcs in tabs.items()
        }
    return tabs


bacc.get_activation_tables = _pinned_act_tables

P = 128
S = 1024          # sequence length (per core)
D = 512           # model dim
H = 8             # heads
HD = 64           # head dim
DFF = 2048        # ffn hidden
DC = D // P       # 4 chunks of model dim
ST = S // P       # 8 tiles of sequence
SC = S // 512     # 2 free-dim chunks of 512
FC = DFF // P     # 16 chunks of ffn dim
EPS = 1e-5
NEG = -1e9
FP = mybir.dt.float32
BF = mybir.dt.bfloat16

NCORES = 8

# bisection/perf switches (env-overridable)
FAST_RECIP = bool(int(os.environ.get("K_FAST_RECIP", "0")))
GPSIMD_DIV = bool(int(os.environ.get("K_GPSIMD_DIV", "0")))
BVB_BF16 = bool(int(os.environ.get("K_BVB_BF16", "0")))


def build(nc):
    AF = mybir.ActivationFunctionType
    ALU = mybir.AluOpType

    # ---------------- DRAM parameters ----------------
    def din(name, shape, dt=FP):
        return nc.dram_tensor(name, shape, dt, kind="ExternalInput").ap()

    xd = din("x", [D, S], BF)          # host passes x.T
    fd = din("feature", [D, S], BF)    # host passes feature.T
    wq_d, bq_d = din("wq", [D, D], BF), din("bq", [D])
    wk_d, bk_d = din("wk", [D, D], BF), din("bk", [D])
    wv_d, bv_d = din("wv", [D, D], BF), din("bv", [D])
    wo_d, bo_d = din("wo", [D, D], BF), din("bo", [D])
    ln1_g_d, ln1_b_d = din("ln1_g", [D]), din("ln1_b", [D])
    wqc_d, bqc_d = din("wqc", [D, D], BF), din("bqc", [D])
    wkc_d, bkc_d = din("wkc", [D, D], BF), din("bkc", [D])
    wvc_d, bvc_d = din("wvc", [D, D], BF), din("bvc", [D])
    woc_d, boc_d = din("woc", [D, D], BF), din("boc", [D])
    ln2_g_d, ln2_b_d = din("ln2_g", [D]), din("ln2_b", [D])
    w1_d, b1_d = din("w1", [D, DFF], BF), din("b1", [DFF])
    w2_d, b2_d = din("w2", [DFF, D], BF), din("b2", [D])
    lnf_g_d, lnf_b_d = din("lnf_g", [D]), din("lnf_b", [D])
    out_d = nc.dram_tensor("out", [D, S], BF, kind="ExternalOutput").ap()

    reps = int(os.environ.get("KERNEL_REPS", "1"))
    with tile.TileContext(nc) as tc, ExitStack() as top:
        const = top.enter_context(tc.tile_pool(name="const", bufs=1))
        chain = top.enter_context(tc.tile_pool(name="chain", bufs=1))
        acts = top.enter_context(tc.tile_pool(name="acts", bufs=1))
        wpool = top.enter_context(tc.tile_pool(name="wpool", bufs=1))
        work = top.enter_context(tc.tile_pool(name="work", bufs=1))
        psum = top.enter_context(tc.tile_pool(name="psum", bufs=1, space="PSUM"))

        def ps_tile(tag, name):
            if tag == "s":
                return psum.tile([P, 2, 512], FP, tag="s", name=name, bufs=2)
            bufs = {"pv": 2, "proj": 2}[tag]
            return psum.tile([P, 512], FP, tag=tag, name=name, bufs=bufs)

        # ---------------- constants ----------------
        # doubled causal tri mask for the diagonal 128-block of a head pair:
        # Z2[p, e, i] = NEG where kv_row p > q_col i (same for both heads).
        Z2 = const.tile([P, 2, 128], FP, tag="Z2", name="Z2")
        nc.gpsimd.memset(Z2[:, :, :], 0.0)
        for e in range(2):
            nc.gpsimd.affine_select(
                out=Z2[:, e, :],
                in_=Z2[:, e, :],
                compare_op=ALU.is_ge,
                fill=NEG,
                base=0,
                pattern=[[1, 128]],
                channel_multiplier=-1,
            )

        ones_col_bf = const.tile([P, 1], BF, tag="ones_col_bf", name="ones_col_bf")
        nc.vector.memset(ones_col_bf[:, :], 1.0)
        # [P, 33] one-hot columns: route the mean reduction to psum row 0 and
        # the meansq reduction to row 32 of the SAME bank (other rows get +0);
        # partition 32 keeps later SBUF reads 32-aligned
        ones_r0 = const.tile([P, 33], BF, tag="ones_r0", name="ones_r0")
        nc.vector.memset(ones_r0[:, :], 0.0)
        nc.vector.memset(ones_r0[:, 0:1], 1.0)
        ones_r32 = const.tile([P, 33], BF, tag="ones_r32", name="ones_r32")
        nc.vector.memset(ones_r32[:, :], 0.0)
        nc.vector.memset(ones_r32[:, 32:33], 1.0)
        eps_col = const.tile([P, 1], FP, tag="eps_col", name="eps_col")
        nc.vector.memset(eps_col[:, :], EPS)

        def big(pool, tag, name, bufs=None, dt=BF):
            return pool.tile([P, DC, S], dt, tag=tag, name=name, bufs=bufs)

        # ---------------- loaders ----------------
        def load_w(dram, K, N, tag, uid, bufs=2):
            t = wpool.tile([P, K // P, N], BF, tag=tag, name=f"{tag}_{uid}", bufs=bufs)
            nc.sync.dma_start(t[:], dram.rearrange("(c p) n -> p c n", p=P))
            return t

        def load_bias_part(dram, K, tag, uid, bufs=2):
            t = wpool.tile([P, K // P], FP, tag=tag, name=f"{tag}_{uid}", bufs=bufs)
            nc.sync.dma_start(t[:], dram.rearrange("(c p) -> p c", p=P))
            return t

        def load_bias_bcast(dram, N, tag, uid):
            """[N] dram bias -> [P, N] broadcast tile (for free-dim bias)."""
            bdt = BF if BVB_BF16 else FP
            t32 = wpool.tile([1, N], FP, tag=tag + "32", name=f"{tag}32_{uid}", bufs=2)
            nc.sync.dma_start(t32[:], dram.rearrange("(a n) -> a n", a=1))
            src_t = t32
            if BVB_BF16:
                t = wpool.tile([1, N], BF, tag=tag + "bf", name=f"{tag}bf_{uid}", bufs=2)
                nc.vector.tensor_copy(t[:, :], t32[:, :])
                src_t = t
            tb = wpool.tile([P, N], bdt, tag=tag + "b", name=f"{tag}b_{uid}", bufs=2)
            nc.gpsimd.partition_broadcast(tb[:, :], src_t[:, :])
            return tb

        def load_T(dram_ap, name, split=False):
            """[D, S] dram (already transposed on host) -> [P, DC, S] sbuf."""
            dst = big(chain, "io", name, bufs=2)
            if split:
                for sc in range(SC):
                    sl = slice(sc * 512, (sc + 1) * 512)
                    nc.sync.dma_start(
                        dst[:, :, sl],
                        dram_ap[:, sl].rearrange("(c p) s -> p c s", p=P),
                    )
            else:
                nc.sync.dma_start(dst[:], dram_ap.rearrange("(c p) s -> p c s", p=P))
            return dst

        def linear_T(in_T, w_sb, bias_part, outT, name, res_T=None):
            """outT[P, DC, S] = w^T @ in_T + bias (+ res_T).  T layout."""
            for sc in range(SC):
                for m in range(DC):
                    ps = ps_tile("proj", f"ps_{name}_{m}_{sc}")
                    for c in range(DC):
                        nc.tensor.matmul(
                            ps[:],
                            lhsT=w_sb[:, c, m * P : (m + 1) * P],
                            rhs=in_T[:, c, sc * 512 : (sc + 1) * 512],
                            start=(c == 0),
                            stop=(c == DC - 1),
                        )
                    o = outT[:, m, sc * 512 : (sc + 1) * 512]
                    if res_T is None:
                        nc.vector.tensor_scalar(
                            o, ps[:], bias_part[:, m : m + 1], None, ALU.add
                        )
                    else:
                        # one fused op: (psum + bias) + residual
                        nc.vector.scalar_tensor_tensor(
                            o, ps[:], bias_part[:, m : m + 1],
                            res_T[:, m, sc * 512 : (sc + 1) * 512],
                            ALU.add, ALU.add,
                        )
            return outT

        def qk_chunk(m, xqT, kvT, wq_sb, bq_sb, wk_sb, bk_sb, QT, KT, name):
            """Output chunk m of both Q and K projections (all sc)."""
            for t, w_sb, b_sb, inT, outT in (
                ("q", wq_sb, bq_sb, xqT, QT),
                ("k", wk_sb, bk_sb, kvT, KT),
            ):
                for sc in range(SC):
                    ps = ps_tile("proj", f"ps_{name}{t}_{m}_{sc}")
                    for c in range(DC):
                        nc.tensor.matmul(
                            ps[:],
                            lhsT=w_sb[:, c, m * P : (m + 1) * P],
                            rhs=inT[:, c, sc * 512 : (sc + 1) * 512],
                            start=(c == 0),
                            stop=(c == DC - 1),
                        )
                    nc.vector.tensor_scalar(
                        outT[:, m, sc * 512 : (sc + 1) * 512],
                        ps[:], b_sb[:, m : m + 1], None, ALU.add,
                    )

        def v_chunk(kt, kvT, wv_sb, bvb, V, name):
            """One 128-kv-token group of V (natural layout + ones column)."""
            ps = ps_tile("proj", f"ps_{name}_{kt}")
            for c in range(DC):
                nc.tensor.matmul(
                    ps[:],
                    lhsT=kvT[:, c, kt * P : (kt + 1) * P],
                    rhs=wv_sb[:, c, :],
                    start=(c == 0),
                    stop=(c == DC - 1),
                )
            # PSUM -> SBUF move with the (free-dim) bias folded in
            nc.vector.tensor_tensor(
                V[:, kt, :, 0:HD],
                ps[:].rearrange("p (h d) -> p h d", h=H),
                bvb.rearrange("p (h d) -> p h d", h=H),
                ALU.add,
            )

        def softmax_hp(sc, hp, QT, KT, V, OT, causal, blk):
            """Scores+softmax+PV for one (q-chunk, head-pair)."""
            qsl0 = sc * 512
            n_kv = (4 * sc + 4) if causal else ST
            pvs = [ps_tile("pv", f"pv{blk}_{hp}_{sc}_{e}") for e in range(2)]
            for j in range(n_kv):
                diag = causal and j >= 4 * sc
                u = j - 4 * sc
                off = 128 * u if diag else 0
                sps = ps_tile("s", f"s{blk}_{hp}_{sc}_{j}")
                for e in range(2):
                    bp = e * 64
                    # paired heads at base partitions 0/64: row-tiled
                    # matmuls run concurrently in the PE array
                    nc.tensor.matmul(
                        sps[:, e, off:512],
                        lhsT=KT[bp : bp + 64, hp, j * P : (j + 1) * P],
                        rhs=QT[bp : bp + 64, hp, qsl0 + off : qsl0 + 512],
                        start=True, stop=True,
                    )
                if diag:
                    nc.vector.tensor_add(
                        sps[:, :, off : off + 128],
                        sps[:, :, off : off + 128],
                        Z2[:, :, :],
                    )
                pT = work.tile([P, 2, 512], BF, tag="pT",
                               name=f"pT{blk}_{hp}_{sc}_{j}", bufs=4)
                nc.scalar.activation(
                    pT[:, :, off:512], sps[:, :, off:512], AF.Exp
                )
                for e in range(2):
                    nc.tensor.matmul(
                        pvs[e][0 : HD + 1, off:512],
                        lhsT=V[:, j, 2 * hp + e, :],
                        rhs=pT[:, e, off:512],
                        start=(j == 0),
                        stop=(j == n_kv - 1),
                    )
            for e in range(2):
                bp = e * 64
                # evacuate the PV bank with one copy; divide off the
                # critical path (recip on DVE, bcast on gpsimd)
                tmp = work.tile([HD + 1, 512], BF, tag="otmp",
                                name=f"otmp{blk}_{hp}_{sc}_{e}", bufs=2)
                nc.vector.tensor_copy(tmp[:, :], pvs[e][0 : HD + 1, :])
                rec = work.tile([1, 512], FP, tag="rec",
                                name=f"rc{blk}_{hp}_{sc}_{e}", bufs=2)
                if FAST_RECIP:
                    nc.vector.reciprocal_approx_fast(rec[:, :], tmp[HD : HD + 1, :])
                else:
                    nc.vector.reciprocal(rec[:, :], tmp[HD : HD + 1, :])
                reprow = work.tile([64, 512], FP, tag="reprow",
                                   name=f"rr{blk}_{hp}_{sc}_{e}", bufs=2)
                nc.gpsimd.partition_broadcast(reprow[:, :], rec[:, :])
                ot_sl = OT[bp : bp + 64, hp, qsl0 : qsl0 + 512]
                if GPSIMD_DIV:
                    nc.gpsimd.tensor_tensor(ot_sl, tmp[0:HD, :], reprow[:, :],
                                            ALU.mult)
                else:
                    nc.vector.tensor_tensor(ot_sl, tmp[0:HD, :], reprow[:, :],
                                            ALU.mult)

        def layernorm_T(inT, g_sb, b_sb, outT, name):
            """LN over the feature dim (partitions x DC chunks), T layout."""
            for sc in range(SC):
                ln_sc(inT, g_sb, b_sb, outT, sc, name)
            return outT

        def ln_sc(inT, g_sb, b_sb, outT, sc, name):
            ln_piece(inT, g_sb, b_sb, outT, sc * 512, 512, f"{name}_{sc}")

        def ln_piece(inT, g_sb, b_sb, outT, q0, W, name):
            if True:
                sl = slice(q0, q0 + W)
                # mean into psum row 0, meansq into row 1 (ones_pair routes
                # the reduction to row 1; row 0 accumulates +0) — one bank
                psAB = ps_tile("pv", f"lnAB_{name}")
                for c in range(DC):
                    nc.tensor.matmul(
                        psAB[0:33, 0:W], lhsT=ones_r0[:, :], rhs=inT[:, c, sl],
                        start=(c == 0), stop=False,
                    )
                for c in range(DC):
                    sq = work.tile([P, 512], BF, tag="sq",
                                   name=f"lnsq_{name}_{c}", bufs=3)
                    nc.vector.tensor_tensor(
                        sq[:, 0:W], inT[:, c, sl], inT[:, c, sl], ALU.mult
                    )
                    nc.tensor.matmul(
                        psAB[0:33, 0:W], lhsT=ones_r32[:, :], rhs=sq[:, 0:W],
                        start=False, stop=(c == DC - 1),
                    )

                def small(tag, dt=FP):
                    t = work.tile([1, 512], dt, tag=tag,
                                  name=f"ln{tag}_{name}", bufs=2)
                    return t[:, 0:W]

                mub = small("mub", BF)
                nc.vector.tensor_scalar_mul(mub[:, :], psAB[0:1, 0:W], 1.0 / D)
                ex2 = small("ex2", BF)
                nc.vector.tensor_scalar_mul(ex2[:, :], psAB[32:33, 0:W], 1.0 / D)
                musq = small("musq", BF)
                nc.vector.tensor_tensor(musq[:, :], mub[:, :], mub[:, :], ALU.mult)
                # ex2 becomes the variance in place
                nc.vector.tensor_tensor(
                    ex2[:, :], ex2[:, :], musq[:, :], ALU.subtract
                )
                # rs = 1/sqrt(var+eps) = exp(-0.5*ln(var+eps)); keeps ACT in
                # the natural_log_exp set (no table reloads)
                lnv = small("lnv", BF)
                nc.scalar.activation(lnv[:, :], ex2[:, :], AF.Ln,
                                     bias=eps_col[0:1, :])
                rs = small("rs", BF)
                nc.scalar.activation(rs[:, :], lnv[:, :], AF.Exp, scale=-0.5)
                murs = small("murs", BF)
                nc.vector.tensor_tensor(murs[:, :], mub[:, :], rs[:, :], ALU.mult)
                rs_rep = work.tile([P, 512], BF, tag="rs_rep",
                                   name=f"lnrsrep_{name}", bufs=2)
                nc.gpsimd.partition_broadcast(rs_rep[:, 0:W], rs[:, :])
                murs_rep = work.tile([P, 512], BF, tag="murs_rep",
                                     name=f"lnmursrep_{name}", bufs=2)
                nc.gpsimd.partition_broadcast(murs_rep[:, 0:W], murs[:, :])
                for c in range(DC):
                    t1 = work.tile([P, 512], BF, tag="t1",
                                   name=f"lnt1_{name}_{c}", bufs=2)
                    nc.vector.tensor_tensor(
                        t1[:, 0:W], inT[:, c, sl], rs_rep[:, 0:W], ALU.mult
                    )
                    nc.vector.tensor_tensor(
                        t1[:, 0:W], t1[:, 0:W], murs_rep[:, 0:W], ALU.subtract
                    )
                    # out = (t1 * g) + b, one DVE op
                    nc.vector.tensor_scalar(
                        outT[:, c, sl], t1[:, 0:W], g_sb[:, c : c + 1],
                        b_sb[:, c : c + 1], ALU.mult, ALU.add,
                    )

        def lin_chunk(in_T, w_sb, bias_part, outT, sc, m, name, res_T=None):
            """One (sc, m) output chunk of a projection."""
            ps = ps_tile("proj", f"ps_{name}_{m}_{sc}")
            for c in range(DC):
                nc.tensor.matmul(
                    ps[:],
                    lhsT=w_sb[:, c, m * P : (m + 1) * P],
                    rhs=in_T[:, c, sc * 512 : (sc + 1) * 512],
                    start=(c == 0),
                    stop=(c == DC - 1),
                )
            o = outT[:, m, sc * 512 : (sc + 1) * 512]
            if res_T is None:
                nc.vector.tensor_scalar(
                    o, ps[:], bias_part[:, m : m + 1], None, ALU.add
                )
            else:
                nc.vector.scalar_tensor_tensor(
                    o, ps[:], bias_part[:, m : m + 1],
                    res_T[:, m, sc * 512 : (sc + 1) * 512],
                    ALU.add, ALU.add,
                )

        # ================ program ================
        for _rep in range(reps):
            _u = "" if _rep == 0 else None  # weights loaded once, reused
            if _rep == 0:
                # first q-projection weight ahead of the activations in the
                # DMA queue: the first matmul needs wq AND x's first half
                wq_sb0 = load_w(wq_d, D, D, "wq", f"s{_rep}")
                bq_sb0 = load_bias_part(bq_d, D, "bq", f"s{_rep}")
            xT = load_T(xd, f"xT{_rep}", split=True)

            if _rep == 0:
             ws0 = ws = (
                wq_sb0,
                bq_sb0,
                load_w(wk_d, D, D, "wk", f"s{_rep}"),
                load_bias_part(bk_d, D, "bk", f"s{_rep}"),
                load_w(wv_d, D, D, "wv", f"s{_rep}"),
                load_bias_bcast(bv_d, D, "bv", f"s{_rep}"),
                load_w(wo_d, D, D, "wo", f"s{_rep}"),
                load_bias_part(bo_d, D, "bo", f"s{_rep}"),
            )
             g1 = load_bias_part(ln1_g_d, D, "lng", f"1{_rep}", bufs=3)
             b1n = load_bias_part(ln1_b_d, D, "lnb", f"1{_rep}", bufs=3)
             wc0 = wc = (
                load_w(wqc_d, D, D, "wq", f"c{_rep}"),
                load_bias_part(bqc_d, D, "bq", f"c{_rep}"),
                load_w(wkc_d, D, D, "wk", f"c{_rep}"),
                load_bias_part(bkc_d, D, "bk", f"c{_rep}"),
                load_w(wvc_d, D, D, "wv", f"c{_rep}"),
                load_bias_bcast(bvc_d, D, "bv", f"c{_rep}"),
                load_w(woc_d, D, D, "wo", f"c{_rep}"),
                load_bias_part(boc_d, D, "bo", f"c{_rep}"),
            )
             g2 = load_bias_part(ln2_g_d, D, "lng", f"2{_rep}", bufs=3)
             b2n = load_bias_part(ln2_b_d, D, "lnb", f"2{_rep}", bufs=3)
             w1_sb = wpool.tile([P, DC, DFF], BF, tag="w1", name=f"w1_{_rep}", bufs=1)
             nc.sync.dma_start(w1_sb[:], w1_d.rearrange("(c p) n -> p c n", p=P))
             w2_sb = wpool.tile([P, FC, D], BF, tag="w2", name=f"w2_{_rep}", bufs=1)
             nc.sync.dma_start(w2_sb[:], w2_d.rearrange("(c p) n -> p c n", p=P))
             b1_sb = load_bias_part(b1_d, DFF, "b1", f"f{_rep}", bufs=1)
             b2_sb = load_bias_part(b2_d, D, "b2", f"f{_rep}", bufs=1)
             gf = load_bias_part(lnf_g_d, D, "lng", f"f{_rep}", bufs=3)
             bf = load_bias_part(lnf_b_d, D, "lnb", f"f{_rep}", bufs=3)
            fT = load_T(fd, f"fT{_rep}")

            # ---------------- self attention ----------------
            (wq_sb, bq_sb, wk_sb, bk_sb, wv_sb, bvb, wo_sb, bo_sb) = ws
            blk = f"s{_rep}"
            QT = big(acts, "qk", f"QT{blk}", bufs=3)
            KT = big(acts, "qk", f"KT{blk}", bufs=3)
            V = acts.tile([P, ST, H, HD + 1], BF, tag="v", name=f"V{blk}", bufs=2)
            nc.vector.memset(V[:, :, :, HD], 1.0)
            OT = big(acts, "o", f"OT{blk}", bufs=2)

            # interleave projection chunks with softmax head-pair loops so
            # exp work starts as early as possible
            qk_chunk(0, xT, xT, wq_sb, bq_sb, wk_sb, bk_sb, QT, KT, blk)
            for kt in range(4):
                v_chunk(kt, xT, wv_sb, bvb, V, f"V{blk}")
            softmax_hp(0, 0, QT, KT, V, OT, True, blk)
            qk_chunk(1, xT, xT, wq_sb, bq_sb, wk_sb, bk_sb, QT, KT, blk)
            for kt in range(4, 8):
                v_chunk(kt, xT, wv_sb, bvb, V, f"V{blk}")
            softmax_hp(0, 1, QT, KT, V, OT, True, blk)
            qk_chunk(2, xT, xT, wq_sb, bq_sb, wk_sb, bk_sb, QT, KT, blk)
            softmax_hp(0, 2, QT, KT, V, OT, True, blk)
            qk_chunk(3, xT, xT, wq_sb, bq_sb, wk_sb, bk_sb, QT, KT, blk)
            softmax_hp(0, 3, QT, KT, V, OT, True, blk)
            # cross-attention K/V are independent of h1 — interleave into the
            # remaining self-softmax loops so PE back-fills ACT-bound stretches
            (wqc_sb, bqc_sb, wkc_sb, bkc_sb, wvc_sb, bvcb, woc_sb, boc_sb) = wc
            blkc = f"c{_rep}"
            KTc = big(acts, "qk", f"KT{blkc}", bufs=3)
            Vc = acts.tile([P, ST, H, HD + 1], BF, tag="v", name=f"V{blkc}", bufs=2)
            nc.vector.memset(Vc[:, :, :, HD], 1.0)

            def kc_chunk(m):
                for sc in range(SC):
                    ps = ps_tile("proj", f"ps_{blkc}k_{m}_{sc}")
                    for c in range(DC):
                        nc.tensor.matmul(
                            ps[:],
                            lhsT=wkc_sb[:, c, m * P : (m + 1) * P],
                            rhs=fT[:, c, sc * 512 : (sc + 1) * 512],
                            start=(c == 0),
                            stop=(c == DC - 1),
                        )
                    nc.vector.tensor_scalar(
                        KTc[:, m, sc * 512 : (sc + 1) * 512],
                        ps[:], bkc_sb[:, m : m + 1], None, ALU.add,
                    )

            for hp in range(4):
                softmax_hp(1, hp, QT, KT, V, OT, True, blk)
                kc_chunk(hp)

            # pipeline the self O-projection + LN1 per 512-token chunk, then
            # cross attention with its O-projection interleaved into the
            # second softmax sweep, then FFN + final LN per chunk
            pre1 = big(chain, "h", f"pre{blk}", bufs=2)
            h1T = big(chain, "h", f"h{blk}", bufs=2)
            for kt in range(ST):
                v_chunk(kt, fT, wvc_sb, bvcb, Vc, f"V{blkc}")
            for m in range(DC):
                lin_chunk(OT, wo_sb, bo_sb, pre1, 0, m, f"pre{blk}", res_T=xT)
            ln_sc(pre1, g1, b1n, h1T, 0, f"h{blk}")

            # ---------------- cross attention ----------------
            # sc0 q-chunks depend only on h1T[:, :, 0:512]; emitting them
            # before the self sc1 O-projection lets the first cross softmax
            # sweep overlap the whole self tail
            QTc = big(acts, "qk", f"QT{blkc}", bufs=3)
            OTc = big(acts, "o", f"OT{blkc}", bufs=2)

            def qtc_chunk(m, sc):
                ps = ps_tile("proj", f"ps_{blkc}q_{m}_{sc}")
                for c in range(DC):
                    nc.tensor.matmul(
                        ps[:],
                        lhsT=wqc_sb[:, c, m * P : (m + 1) * P],
                        rhs=h1T[:, c, sc * 512 : (sc + 1) * 512],
                        start=(c == 0),
                        stop=(c == DC - 1),
                    )
                nc.vector.tensor_scalar(
                    QTc[:, m, sc * 512 : (sc + 1) * 512],
                    ps[:], bqc_sb[:, m : m + 1], None, ALU.add,
                )

            for m in range(DC):
                lin_chunk(OT, wo_sb, bo_sb, pre1, 1, m, f"pre{blk}", res_T=xT)
            ln_sc(pre1, g1, b1n, h1T, 1, f"h{blk}")
            for m in range(DC):
                qtc_chunk(m, 0)
            for m in range(DC):
                softmax_hp(0, m, QTc, KTc, Vc, OTc, False, blkc)
                qtc_chunk(m, 1)
            pre2 = big(chain, "h", f"pre{blkc}", bufs=2)
            h2T = big(chain, "h", f"h{blkc}", bufs=2)
            for hp in range(4):
                softmax_hp(1, hp, QTc, KTc, Vc, OTc, False, blkc)
                lin_chunk(OTc, woc_sb, boc_sb, pre2, 0, hp, f"pre{blkc}",
                          res_T=h1T)
            ln_sc(pre2, g2, b2n, h2T, 0, f"h{blkc}")
            for m in range(DC):
                lin_chunk(OTc, woc_sb, boc_sb, pre2, 1, m, f"pre{blkc}",
                          res_T=h1T)
            ln_sc(pre2, g2, b2n, h2T, 1, f"h{blkc}")

            # ---------------- FFN ----------------
            # sc1 runs in two 256-token chunks so the final-LN pieces for the
            # third quarter start while the last quarter is still computing
            pre3 = big(chain, "h", f"pre3_{_rep}", bufs=2)
            for q0, W in ((0, 512), (512, 256), (768, 256)):
                sl = slice(q0, q0 + W)
                # 4 held accumulators live in the halves of two 2-bank s tiles
                ff2t = [
                    ps_tile("s", f"ff2_{_rep}_{q0}_0"),
                    ps_tile("s", f"ff2_{_rep}_{q0}_1"),
                ]
                ff2ps = [ff2t[m // 2][:, m % 2, 0:W] for m in range(DC)]
                for f in range(FC):
                    fps = ps_tile("proj", f"ff1_{_rep}_{q0}_{f}")
                    for c in range(DC):
                        nc.tensor.matmul(
                            fps[:, 0:W],
                            lhsT=w1_sb[:, c, f * P : (f + 1) * P],
                            rhs=h2T[:, c, sl],
                            start=(c == 0),
                            stop=(c == DC - 1),
                        )
                    ff1 = work.tile([P, 512], BF, tag="ff1",
                                    name=f"ff1sb_{_rep}_{q0}_{f}", bufs=3)
                    nc.scalar.activation(
                        ff1[:, 0:W], fps[:, 0:W], AF.Relu,
                        bias=b1_sb[:, f : f + 1]
                    )
                    for m in range(DC):
                        nc.tensor.matmul(
                            ff2ps[m],
                            lhsT=w2_sb[:, f, m * P : (m + 1) * P],
                            rhs=ff1[:, 0:W],
                            start=(f == 0),
                            stop=(f == FC - 1),
                        )
                for m in range(DC):
                    # one fused op: (psum + bias) + residual
                    nc.vector.scalar_tensor_tensor(
                        pre3[:, m, sl], ff2ps[m], b2_sb[:, m : m + 1],
                        h2T[:, m, sl], ALU.add, ALU.add,
                    )
            outT = big(chain, "io", f"outT{_rep}", bufs=2)
            for q0 in range(0, S, 256):
                ln_piece(pre3, gf, bf, outT, q0, 256, f"outT{_rep}_{q0}")

            # ---------------- output DMA (transposed bf16; host converts) ----------------
            for m in range(DC):
                for sc in range(SC):
                    nc.sync.dma_start(
                        out_d[m * P : (m + 1) * P, sc * 512 : (sc + 1) * 512]
                        .rearrange("(a p) s -> p a s", a=1),
                        outT[:, m : m + 1, sc * 512 : (sc + 1) * 512],
                    )

    return nc


_CACHE = {}


def _get_graph():
    if "nc" not in _CACHE:
        nc = bacc.Bacc(
            "TRN2", target_bir_lowering=False, debug=False, num_devices=NCORES
        )
        build(nc)
        nc.compile()
        _CACHE["nc"] = nc
    return _CACHE["nc"]


def kernel(**inputs):
    nc = _get_graph()
    scale = 1.0 / np.sqrt(np.float32(D))

    BF_NP = ml_dtypes.bfloat16
    BF_KEYS = {"wq", "wk", "wv", "wo", "wqc", "wkc", "wvc", "woc", "w1", "w2"}
    weights = {}
    for k, v in inputs.items():
        if k in ("x", "feature"):
            continue
        weights[k] = np.ascontiguousarray(np.asarray(v, dtype=np.float32))
    # fold the 1/sqrt(D) score scaling into the query projections
    for k in ("wq", "bq", "wqc", "bqc"):
        weights[k] = weights[k] * scale
    for k in BF_KEYS:
        weights[k] = weights[k].astype(BF_NP)

    x = np.asarray(inputs["x"], dtype=np.float32).astype(BF_NP)
    feature = np.asarray(inputs["feature"], dtype=np.float32).astype(BF_NP)

    in_maps = []
    for i in range(NCORES):
        m = dict(weights)
        m["x"] = np.ascontiguousarray(x[i].T)
        m["feature"] = np.ascontiguousarray(feature[i].T)
        in_maps.append(m)

    trace = bool(int(os.environ.get("KERNEL_TRACE", "0")))
    kw = {}
    if trace:
        kw["trace"] = True
        kw["tmpdir"] = os.environ.get("KERNEL_TRACE_DIR") or None
    res = run_bass_kernel_spmd(nc, in_maps, core_ids=list(range(NCORES)), **kw)
    if trace:
        print(f"HW exec time: {res.exec_time_ns} ns")
        _CACHE["exec_time_ns"] = res.exec_time_ns
    out = np.stack(
        [res.results[i]["out"].astype(np.float32).T for i in range(NCORES)], axis=0
    )
    return np.ascontiguousarray(out), inputs["feature"]


if __name__ == "__main__":
    _get_graph()
    print("graph built OK")
